# revision 1
# baseline (speedup 1.0000x reference)
"""Trainium2 Bass kernel for nn_NnBoard768 (sparse embedding lookup NNUE head).

Strategy (data-parallel over batch, 8 cores):
  - Each core handles 1024 of the 8192 batch rows. Batch row b sits at
    SBUF partition b%128, free-slot b//128.
  - The feature table is shipped to HBM premultiplied by TSCALE in fp8
    (e4m3), quartering gather traffic; the PE accumulates in fp32 and its
    identity is diag(1/TSCALE), so only the table entries round.
  - Rows are fetched with the TIE-accelerated `dma_gather` instruction
    (4 SWDGE queues; descriptor generation is the main serial cost).
    Its indices are int16, so each (side, k) gather runs as two passes:
    pass A covers table rows < SA, pass B covers the rest (rebased).
    Out-of-pass slots read a zero row from a 4096-row zero block in
    front of the pass base (spread across rows to avoid hammering one
    HBM channel), so every index is valid, every slot is written, and
    invalid-slot reads contribute nothing to the sum.
  - The sum over the 32 active features runs on the tensor engine:
    scaled-identity matmuls accumulate every gathered tile into PSUM.
  - Epilogue on DVE/ACT: +b_ft, clip(0,1), dot with W_out, +b_out, sigmoid.
"""

import sys

sys.path.insert(0, "/opt/trn_rl_repo")

import numpy as np
import ml_dtypes

from concourse import bacc, bass, mybir
from concourse.masks import make_identity
import concourse.tile as tile
from concourse.bass_utils import run_bass_kernel_spmd

P = 128          # SBUF partitions
K = 32           # nnz (active features per position)
J = 8            # batch slots per partition
F = 512          # feature-table output width
NCORES = 8
BPC = P * J      # batch rows per core (1024)
FT_IN = 40960
ZPAD = 4096      # zero rows in front of each pass base (junk reads spread
                 # across them instead of hammering one HBM row)
SA = 32768 - ZPAD          # rows < SA -> pass A; rest -> pass B
# device table layout: [ZA(ZPAD), W[0:SA], ZB(ZPAD), W[SA:]]
VDEV = FT_IN + 2 * ZPAD
BOFF = ZPAD + SA           # byte row where the B view starts (= 32768)
S16 = BPC // 16  # columns of the 16-partition-wrapped index tile (64)

f32 = mybir.dt.float32
bf16 = mybir.dt.bfloat16
i16 = mybir.dt.int16
Alu = mybir.AluOpType

TDT = mybir.dt.float8e4            # gathered-table dtype on device
TDT_NP = ml_dtypes.float8_e4m3     # host equivalent
TSCALE = 64.0                      # host premultiplier; PE identity = 1/TSCALE

GBUFS = 8        # in-flight gather tiles per (A/B) tag
NQ = 4           # SWDGE descriptor-generation queues (parallel on HW)


def _build(fast: bool):
    nc = bacc.Bacc("TRN2", target_bir_lowering=False, debug=False, num_devices=NCORES,
                   num_swdge_queues=NQ)

    idx_in = {}
    for side in ("stm", "nstm"):
        for part in ("a", "b"):
            idx_in[(side, part)] = nc.dram_tensor(
                f"i{part}_{side}", [P, K, S16], i16, kind="ExternalInput"
            )
    wft = nc.dram_tensor("w_ft", [VDEV, F], TDT, kind="ExternalInput")
    bft = nc.dram_tensor("bft", [P, F], f32, kind="ExternalInput")
    w1 = nc.dram_tensor("w1", [P, F], f32, kind="ExternalInput")
    w2 = nc.dram_tensor("w2", [P, F], f32, kind="ExternalInput")
    bout = nc.dram_tensor("bout", [P, 1], f32, kind="ExternalInput")
    if not fast:
        vals = nc.dram_tensor("vals", [P, K, J], f32, kind="ExternalInput")
    out = nc.dram_tensor("out", [P, J], f32, kind="ExternalOutput")

    gbufs = GBUFS if fast else 2
    with tile.TileContext(nc) as tc:
        with tc.tile_pool(name="sbuf", bufs=1) as pool, \
             tc.tile_pool(name="gather", bufs=gbufs) as gpool, \
             tc.tile_pool(name="psum", bufs=1, space="PSUM") as ppool:
            idx_sb = {}
            for side_i, side in enumerate(("stm", "nstm")):
                for part in ("a", "b"):
                    t = pool.tile(
                        [P, K, S16], i16,
                        tag=f"i{part}{side_i}", name=f"i{part}_{side}_sb",
                    )
                    nc.sync.dma_start(out=t[:], in_=idx_in[(side, part)][:])
                    idx_sb[(side_i, part)] = t
            bft_sb = pool.tile([P, F], f32, tag="bft", name="bft_sb")
            nc.sync.dma_start(out=bft_sb[:], in_=bft[:])
            w_sb = [
                pool.tile([P, F], f32, tag="w1", name="w1_sb"),
                pool.tile([P, F], f32, tag="w2", name="w2_sb"),
            ]
            nc.sync.dma_start(out=w_sb[0][:], in_=w1[:])
            nc.sync.dma_start(out=w_sb[1][:], in_=w2[:])
            bout_sb = pool.tile([P, 1], f32, tag="bout", name="bout_sb")
            nc.sync.dma_start(out=bout_sb[:], in_=bout[:])
            ident = pool.tile([P, P], TDT, tag="ident", name="ident")
            make_identity(nc, ident[:])
            nc.vector.tensor_scalar_mul(ident[:], ident[:], 1.0 / TSCALE)
            if not fast:
                vals_sb = pool.tile([P, K, J], f32, tag="vals", name="vals_sb")
                nc.sync.dma_start(out=vals_sb[:], in_=vals[:])

            def bcast(t2d):  # [P, F] -> [P, J, F] AP (stride-0 over J)
                return t2d[:].rearrange("p (j f) -> p j f", j=1).broadcast_to([P, J, F])

            z = [
                pool.tile([P, J], f32, tag=f"z{side}", name=f"z{side}")
                for side in range(2)
            ]
            for side in range(2):
                if fast:
                    acc = ppool.tile(
                        [P, J, F], f32, space="PSUM", tag="acc", name=f"acc{side}"
                    )
                else:
                    acc = pool.tile([P, J, F], f32, tag=f"sacc{side}", name=f"sacc{side}")
                for k in range(K):
                    ga = gpool.tile([P, J, F], TDT, tag="ga", name="ga")
                    gb = gpool.tile([P, J, F], TDT, tag="gb", name="gb")
                    qa = (side * 2 * K + 2 * k) % NQ
                    nc.gpsimd.dma_gather(
                        ga[:], wft[:, :], idx_sb[(side, "a")][:, k, :],
                        num_idxs=BPC, num_idxs_reg=BPC, elem_size=F,
                        queue_num=qa,
                    )
                    nc.gpsimd.dma_gather(
                        gb[:], wft[BOFF:, :], idx_sb[(side, "b")][:, k, :],
                        num_idxs=BPC, num_idxs_reg=BPC, elem_size=F,
                        queue_num=(qa + 1) % NQ,
                    )
                    if fast:
                        for j in range(J):
                            nc.tensor.matmul(
                                acc[:, j, :], ident[:], ga[:, j, :],
                                start=(k == 0), stop=False,
                            )
                        for j in range(J):
                            nc.tensor.matmul(
                                acc[:, j, :], ident[:], gb[:, j, :],
                                start=False, stop=(k == K - 1),
                            )
                    else:
                        vb = (
                            vals_sb[:, k, :]
                            .rearrange("p (j f) -> p j f", f=1)
                            .broadcast_to([P, J, F])
                        )
                        t = gpool.tile([P, J, F], f32, tag="t", name="t")
                        nc.vector.tensor_tensor(out=t[:], in0=ga[:], in1=gb[:], op=Alu.add)
                        if k == 0:
                            nc.vector.tensor_tensor(out=acc[:], in0=t[:], in1=vb, op=Alu.mult)
                        else:
                            nc.vector.tensor_tensor(out=t[:], in0=t[:], in1=vb, op=Alu.mult)
                            nc.vector.tensor_tensor(out=acc[:], in0=acc[:], in1=t[:], op=Alu.add)

                # epilogue: h = clip(acc + b_ft, 0, 1) * w_side; z = sum_f h
                h = pool.tile([P, J, F], f32, tag=f"h{side}", name=f"h{side}")
                nc.vector.tensor_tensor(out=h[:], in0=acc[:], in1=bcast(bft_sb), op=Alu.add)
                nc.vector.tensor_scalar(
                    out=h[:], in0=h[:], scalar1=0.0, scalar2=1.0,
                    op0=Alu.max, op1=Alu.min,
                )
                nc.vector.tensor_tensor(out=h[:], in0=h[:], in1=bcast(w_sb[side]), op=Alu.mult)
                nc.vector.tensor_reduce(
                    out=z[side][:], in_=h[:], axis=mybir.AxisListType.X, op=Alu.add
                )
            nc.vector.tensor_tensor(out=z[0][:], in0=z[0][:], in1=z[1][:], op=Alu.add)
            out_sb = pool.tile([P, J], f32, tag="out", name="out_sb")
            nc.scalar.activation(
                out=out_sb[:],
                in_=z[0][:],
                func=mybir.ActivationFunctionType.Sigmoid,
                bias=bout_sb[:, :1],
            )
            nc.sync.dma_start(out=out.ap(), in_=out_sb[:])

    nc.compile()
    return nc


_cache = {}


def _get(fast: bool):
    if fast not in _cache:
        _cache[fast] = _build(fast)
    return _cache[fast]


def _prep_table(W_ft: np.ndarray) -> np.ndarray:
    """f32 [40960, 512] -> TSCALE-premultiplied TDT [VDEV, 512]: zero pad
    blocks ahead of each pass segment so junk reads land on spread-out
    zero rows."""
    w = np.zeros((VDEV, F), dtype=TDT_NP)
    w[ZPAD:ZPAD + SA] = (W_ft[:SA] * TSCALE).astype(TDT_NP)
    w[BOFF + ZPAD:] = (W_ft[SA:] * TSCALE).astype(TDT_NP)
    return w


def _prep_idx(idx_core: np.ndarray):
    """[1024, 32] int32 -> (A, B) int16 arrays of shape [128, 32, 64].

    Index g (= batch row b) for feature-slot k lives at partition g%16,
    column g//16 (replicated across the 8 16-partition groups).
    Out-of-pass slots read a (spread) zero row from the pass's ZPAD
    block, so every index is valid and every slot is written.
    """
    t3 = idx_core.astype(np.int64).reshape(S16, 16, K).transpose(2, 1, 0)  # [K,16,S16]
    spread = (np.arange(t3.size, dtype=np.int64).reshape(t3.shape) * 37) % ZPAD
    a = np.where(t3 < SA, t3 + ZPAD, spread).astype(np.int16)
    b = np.where(t3 >= SA, t3 - SA + ZPAD, spread).astype(np.int16)
    a = np.ascontiguousarray(np.tile(a, (1, 8, 1)).transpose(1, 0, 2))  # [128,K,S16]
    b = np.ascontiguousarray(np.tile(b, (1, 8, 1)).transpose(1, 0, 2))
    return a, b


def kernel(stm_indices, nstm_indices, values, W_ft, b_ft, W_out, b_out, _trace=False):
    stm_indices = np.asarray(stm_indices)
    nstm_indices = np.asarray(nstm_indices)
    values = np.asarray(values, dtype=np.float32)
    W_ft = np.ascontiguousarray(np.asarray(W_ft, dtype=np.float32))
    b_ft = np.asarray(b_ft, dtype=np.float32)
    W_out = np.asarray(W_out, dtype=np.float32)
    b_out = np.asarray(b_out, dtype=np.float32)

    fast = bool(np.all(values == 1.0))
    nc = _get(fast)

    w_dev = _prep_table(W_ft)
    bft_rep = np.ascontiguousarray(np.broadcast_to(b_ft, (P, F)).astype(np.float32))
    w1_rep = np.ascontiguousarray(np.broadcast_to(W_out[:F, 0], (P, F)).astype(np.float32))
    w2_rep = np.ascontiguousarray(np.broadcast_to(W_out[F:, 0], (P, F)).astype(np.float32))
    bout_rep = np.full((P, 1), b_out[0], dtype=np.float32)

    in_maps = []
    for c in range(NCORES):
        sl = slice(c * BPC, (c + 1) * BPC)
        m = {
            "w_ft": w_dev,
            "bft": bft_rep,
            "w1": w1_rep,
            "w2": w2_rep,
            "bout": bout_rep,
        }
        for side, arr in (("stm", stm_indices), ("nstm", nstm_indices)):
            a, b = _prep_idx(arr[sl])
            m[f"ia_{side}"] = a
            m[f"ib_{side}"] = b
        if not fast:
            # vals[p, k, j] = values[j*128 + p, k]
            m["vals"] = np.ascontiguousarray(
                values[sl].reshape(J, P, K).transpose(1, 2, 0) / TSCALE
            )
        in_maps.append(m)

    res = run_bass_kernel_spmd(
        nc, in_maps, core_ids=list(range(NCORES)), trace=_trace
    )
    # out[p, j] holds batch row j*128 + p
    out = np.concatenate(
        [res.results[c]["out"].T.reshape(BPC) for c in range(NCORES)]
    ).reshape(8192, 1)
    if _trace:
        return out, res
    return out



# revision 10
# speedup vs baseline: 1.5965x; 1.5965x over previous
"""Trainium2 Bass kernel for nn_NnBoard768 (sparse embedding lookup NNUE head).

Strategy (data-parallel over batch, 8 cores):
  - Each core handles 1024 of the 8192 batch rows. Batch row b sits at
    SBUF partition b%128, free-slot b//128.
  - The core's 1024 rows are split into G=2 groups of 512 rows. For each
    group, the ~22.6k unique feature-table rows referenced by the group's
    32768 draws (2 sides x 512 rows x 32 nnz) are relabeled by first use
    into a compacted per-group table in HBM (fp8 e4m3, premultiplied by
    TSCALE). Labels always fit int16, so every gather is a single pass
    with no junk reads. First-use ordering also makes the first
    occurrence of each row an ascending HBM address in the gather stream.
  - Gathers are batched: one TIE-accelerated `dma_gather` fetches 2048
    rows (4 k-slots x 512 batch rows).  (num_idxs=4096 hangs the HW;
    2048 and below are fine.)
  - The sum over the 32 active features runs on the tensor engine with
    fp8 DoubleRow matmuls: a stacked pair of scaled identities contracts
    TWO gathered k-tiles per matmul (0.5 cycles/row).  b_ft is folded
    into the same PSUM accumulation via a rank-1 bf16 matmul
    (ones[1,128] x b_ft[1,512]).
  - PSUM split: 4 banks per (group, side) accumulator, so side/group
    phases pipeline (epilogue of one phase overlaps matmuls of the next).
  - Epilogue: clip(acc,0,1) -> bf16, * W_out half (bf16), reduce -> f32;
    sigmoid(+b_out) on ACT; all DVE traffic in bf16 to halve cost.
  - Inputs the fast path can't handle (values != 1, or a group with
    >32767 unique rows) fall back to a host-side numpy computation.
"""

import sys

sys.path.insert(0, "/opt/trn_rl_repo")

import numpy as np
import ml_dtypes

from concourse import bacc, bass, mybir
from concourse.masks import make_identity
import concourse.tile as tile
from concourse.bass_utils import run_bass_kernel_spmd

P = 128          # SBUF partitions
K = 32           # nnz (active features per position)
J = 8            # batch slots per partition
F = 512          # feature-table output width
NCORES = 8
BPC = P * J      # batch rows per core (1024)
FT_IN = 40960

G = 2            # index groups per core (512 batch rows each)
JG = J // G      # j-slots per group (4)
KC = 2           # k-slots per gather instruction
NI = (BPC // G) * KC   # indices per gather (1024); >1024 hangs the HW ucode
NGPG = K // KC         # gathers per (group, side)
NGATH = G * 2 * NGPG   # total gathers per core
S16 = NI // 16         # columns of the 16-partition-wrapped index tile

GBUFS = 10       # in-flight gather tiles
NQ = 4           # SWDGE descriptor-generation queues

f32 = mybir.dt.float32
bf16 = mybir.dt.bfloat16
i16 = mybir.dt.int16
Alu = mybir.AluOpType

TDT = mybir.dt.float8e4            # gathered-table dtype on device
TDT_NP = ml_dtypes.float8_e4m3     # host equivalent
BF16_NP = ml_dtypes.bfloat16
TSCALE = 64.0                      # host premultiplier; PE identity = 1/TSCALE
DR = mybir.MatmulPerfMode.DoubleRow
USE_DR = True                      # fp8 DoubleRow matmuls (2 k-tiles per matmul)


def _build(u_max: int):
    nc = bacc.Bacc("TRN2", target_bir_lowering=False, debug=False,
                   num_devices=NCORES, num_swdge_queues=NQ)

    tabs = [
        nc.dram_tensor(f"tab{g}", [u_max, F], TDT, kind="ExternalInput")
        for g in range(G)
    ]
    idx_in = nc.dram_tensor("idx", [P, NGATH, S16], i16, kind="ExternalInput")
    bftb = nc.dram_tensor("bftb", [1, F], bf16, kind="ExternalInput")
    w1 = nc.dram_tensor("w1", [P, F], bf16, kind="ExternalInput")
    w2 = nc.dram_tensor("w2", [P, F], bf16, kind="ExternalInput")
    bout = nc.dram_tensor("bout", [P, 1], f32, kind="ExternalInput")
    out = nc.dram_tensor("out", [P, J], f32, kind="ExternalOutput")

    with tile.TileContext(nc) as tc:
        with tc.tile_pool(name="sbuf", bufs=1) as pool, \
             tc.tile_pool(name="gather", bufs=GBUFS) as gpool, \
             tc.tile_pool(name="psum", bufs=1, space="PSUM") as ppool:
            idx_sb = pool.tile([P, NGATH, S16], i16, tag="idx", name="idx_sb")
            nc.sync.dma_start(out=idx_sb[:], in_=idx_in[:])
            bftb_sb = pool.tile([1, F], bf16, tag="bftb", name="bftb_sb")
            nc.sync.dma_start(out=bftb_sb[:], in_=bftb[:])
            w_sb = [
                pool.tile([P, F], bf16, tag="w1", name="w1_sb"),
                pool.tile([P, F], bf16, tag="w2", name="w2_sb"),
            ]
            nc.sync.dma_start(out=w_sb[0][:], in_=w1[:])
            nc.sync.dma_start(out=w_sb[1][:], in_=w2[:])
            bout_sb = pool.tile([P, 1], f32, tag="bout", name="bout_sb")
            nc.sync.dma_start(out=bout_sb[:], in_=bout[:])

            ones1 = pool.tile([1, P], bf16, tag="ones1", name="ones1")
            nc.gpsimd.memset(ones1[:], 1.0)

            if USE_DR:
                # stacked identity pair for DoubleRow: lhsT[p, t, m] = (p==m)/TSCALE
                ident2 = pool.tile([P, 2, P], TDT, tag="ident", name="ident2")
                make_identity(nc, ident2[:, 0, :])
                make_identity(nc, ident2[:, 1, :])
            else:
                ident2 = pool.tile([P, P], TDT, tag="ident", name="ident")
                make_identity(nc, ident2[:])
            nc.vector.tensor_scalar_mul(ident2[:], ident2[:], 1.0 / TSCALE)

            def bcast(t2d):  # [P, F] -> [P, JG, F] AP (stride-0 over JG)
                return t2d[:].rearrange("p (j f) -> p j f", j=1).broadcast_to([P, JG, F])

            zp = {}
            for g in range(G):
                for s in range(2):
                    acc = ppool.tile(
                        [P, JG, F], f32, space="PSUM", tag=f"acc{s}",
                        name=f"acc{g}{s}",
                    )
                    # open each chain with the bias: acc[m, f] = 1 * b_ft[f]
                    for jg in range(JG):
                        nc.tensor.matmul(
                            acc[:, jg, :], ones1[:], bftb_sb[:],
                            start=True, stop=False, skip_group_check=True,
                        )
                    for kc in range(NGPG):
                        gi = (g * 2 + s) * NGPG + kc
                        ga = gpool.tile([P, NI // P, F], TDT, tag="g", name=f"ga{gi}")
                        nc.gpsimd.dma_gather(
                            ga[:], tabs[g][:, :], idx_sb[:, gi, :],
                            num_idxs=NI, num_idxs_reg=NI, elem_size=F,
                            queue_num=gi % NQ,
                        )
                        if USE_DR:
                            for jg in range(JG):
                                for pt in range(KC // 2):
                                    nc.tensor.matmul(
                                        acc[:, jg, :], ident2[:],
                                        ga[:, jg * KC + 2 * pt: jg * KC + 2 * pt + 2, :],
                                        start=False,
                                        stop=(kc == NGPG - 1 and pt == KC // 2 - 1),
                                        perf_mode=DR,
                                        skip_group_check=True,
                                    )
                        else:
                            for jg in range(JG):
                                for t in range(KC):
                                    nc.tensor.matmul(
                                        acc[:, jg, :], ident2[:],
                                        ga[:, jg * KC + t, :],
                                        start=False,
                                        stop=(kc == NGPG - 1 and t == KC - 1),
                                        skip_group_check=True,
                                    )
                    # epilogue: h = clip(acc, 0, 1) (bf16); h *= w_side; zp = sum_f h
                    h = pool.tile([P, JG, F], bf16, tag=f"h{s}", name=f"h{g}{s}")
                    nc.vector.tensor_scalar(
                        out=h[:], in0=acc[:], scalar1=0.0, scalar2=1.0,
                        op0=Alu.max, op1=Alu.min,
                    )
                    nc.vector.tensor_tensor(out=h[:], in0=h[:], in1=bcast(w_sb[s]), op=Alu.mult)
                    zt = pool.tile([P, JG], f32, tag=f"zp{g}{s}", name=f"zp{g}{s}")
                    nc.vector.tensor_reduce(
                        out=zt[:], in_=h[:], axis=mybir.AxisListType.X, op=Alu.add
                    )
                    zp[(g, s)] = zt

            z = pool.tile([P, J], f32, tag="z", name="z")
            for g in range(G):
                nc.vector.tensor_tensor(
                    out=z[:, g * JG:(g + 1) * JG], in0=zp[(g, 0)][:],
                    in1=zp[(g, 1)][:], op=Alu.add,
                )
            out_sb = pool.tile([P, J], f32, tag="out", name="out_sb")
            nc.scalar.activation(
                out=out_sb[:],
                in_=z[:],
                func=mybir.ActivationFunctionType.Sigmoid,
                bias=bout_sb[:, :1],
            )
            nc.sync.dma_start(out=out.ap(), in_=out_sb[:])

    nc.compile()
    return nc


_cache = {}


def _get(u_max: int):
    if u_max not in _cache:
        _cache[u_max] = _build(u_max)
    return _cache[u_max]


def _prep_group(stm_g: np.ndarray, nstm_g: np.ndarray, W8: np.ndarray):
    """Per (core, group): relabel the group's draws by first use (in gather
    scan order), build the compacted fp8 table and wrapped int16 index tiles.

    stm_g/nstm_g: [512, 32] int32 (group batch rows, core-local order
    r = jg*128 + p). Returns (table [U, F] fp8, idx [2*NGPG, 128, S16] i16, U).
    """
    arrs = []
    for A in (stm_g, nstm_g):
        # [jg, p, kc, t] -> [kc, jg, t, p]; flatten -> gather position order
        B = A.reshape(JG, P, NGPG, KC).transpose(2, 0, 3, 1)
        arrs.append(B.reshape(NGPG, NI))
    S = np.concatenate(arrs, axis=0)          # [2*NGPG, NI] in scan order
    flat = S.ravel().astype(np.int64)
    u, first = np.unique(flat, return_index=True)
    order = np.argsort(first)                 # first-use order of unique rows
    rank = np.empty(len(u), dtype=np.int64)
    rank[order] = np.arange(len(u))
    labels = rank[np.searchsorted(u, flat)]
    if len(u) > 32767:
        return None, None, len(u)
    tab = W8[u[order]]                        # [U, F] fp8
    idx16 = labels.astype(np.int16).reshape(2 * NGPG, S16, 16)
    idxw = np.ascontiguousarray(
        np.tile(idx16.transpose(0, 2, 1), (1, J, 1))
    )                                         # [2*NGPG, 128, S16]
    return tab, idxw, len(u)


def _numpy_fallback(stm_indices, nstm_indices, values, W_ft, b_ft, W_out, b_out):
    stm_ft = np.einsum("bk,bkf->bf", values, W_ft[stm_indices]) + b_ft
    nstm_ft = np.einsum("bk,bkf->bf", values, W_ft[nstm_indices]) + b_ft
    hidden = np.clip(np.concatenate([stm_ft, nstm_ft], axis=1), 0.0, 1.0)
    zv = hidden @ W_out + b_out
    return (1.0 / (1.0 + np.exp(-zv))).astype(np.float32)


def kernel(stm_indices, nstm_indices, values, W_ft, b_ft, W_out, b_out, _trace=False):
    stm_indices = np.asarray(stm_indices)
    nstm_indices = np.asarray(nstm_indices)
    values = np.asarray(values, dtype=np.float32)
    W_ft = np.ascontiguousarray(np.asarray(W_ft, dtype=np.float32))
    b_ft = np.asarray(b_ft, dtype=np.float32)
    W_out = np.asarray(W_out, dtype=np.float32)
    b_out = np.asarray(b_out, dtype=np.float32)

    if not bool(np.all(values == 1.0)):
        r = _numpy_fallback(stm_indices, nstm_indices, values, W_ft, b_ft,
                            W_out, b_out)
        return (r, None) if _trace else r

    W8 = (W_ft * TSCALE).astype(TDT_NP)

    tabs = []   # [core][group] -> table
    idxs = []   # [core] -> [NGATH, 128, S16]
    u_max = 0
    for c in range(NCORES):
        ct, ci = [], []
        for g in range(G):
            sl = slice(c * BPC + g * (BPC // G), c * BPC + (g + 1) * (BPC // G))
            tab, idxw, u = _prep_group(stm_indices[sl], nstm_indices[sl], W8)
            if tab is None:
                r = _numpy_fallback(stm_indices, nstm_indices, values, W_ft,
                                    b_ft, W_out, b_out)
                return (r, None) if _trace else r
            ct.append(tab)
            ci.append(idxw)
            u_max = max(u_max, u)
        tabs.append(ct)
        idxs.append(np.concatenate(ci, axis=0))

    u_pad = -(-u_max // 1024) * 1024
    nc = _get(u_pad)

    bftb_rep = np.ascontiguousarray(b_ft.astype(BF16_NP).reshape(1, F))
    w1_rep = np.ascontiguousarray(
        np.broadcast_to(W_out[:F, 0], (P, F)).astype(BF16_NP))
    w2_rep = np.ascontiguousarray(
        np.broadcast_to(W_out[F:, 0], (P, F)).astype(BF16_NP))
    bout_rep = np.full((P, 1), b_out[0], dtype=np.float32)

    in_maps = []
    for c in range(NCORES):
        m = {
            "idx": np.ascontiguousarray(idxs[c].transpose(1, 0, 2)),  # [P, NGATH, S16]
            "bftb": bftb_rep,
            "w1": w1_rep,
            "w2": w2_rep,
            "bout": bout_rep,
        }
        for g in range(G):
            tfull = np.zeros((u_pad, F), dtype=TDT_NP)
            tfull[:tabs[c][g].shape[0]] = tabs[c][g]
            m[f"tab{g}"] = tfull
        in_maps.append(m)

    res = run_bass_kernel_spmd(
        nc, in_maps, core_ids=list(range(NCORES)), trace=_trace
    )
    # out[p, j] holds batch row j*128 + p
    out = np.concatenate(
        [res.results[c]["out"].T.reshape(BPC) for c in range(NCORES)]
    ).reshape(8192, 1)
    if _trace:
        return out, res
    return out


# revision 14
# speedup vs baseline: 1.9023x; 1.1915x over previous
"""Trainium2 Bass kernel for nn_NnBoard768 (sparse embedding lookup NNUE head).

Strategy (data-parallel over batch, 8 cores):
  - Each core handles 1024 of the 8192 batch rows. Batch row b sits at
    SBUF partition b%128, free-slot b//128.
  - The core's 1024 rows are split into G=2 groups of 512 rows. For each
    group, the ~22.6k unique feature-table rows referenced by the group's
    32768 draws (2 sides x 512 rows x 32 nnz) are relabeled by first use
    into a compacted per-group table in HBM (fp8 e4m3, premultiplied by
    TSCALE). Labels always fit int16, so every gather is a single pass
    with no junk reads. First-use ordering also makes the first
    occurrence of each row an ascending HBM address in the gather stream.
  - Gathers are batched: one TIE-accelerated `dma_gather` fetches 2048
    rows (4 k-slots x 512 batch rows).  (num_idxs=4096 hangs the HW;
    2048 and below are fine.)
  - The sum over the 32 active features runs on the tensor engine with
    fp8 DoubleRow matmuls: a stacked pair of scaled identities contracts
    TWO gathered k-tiles per matmul (0.5 cycles/row).  b_ft is folded
    into the same PSUM accumulation via a rank-1 bf16 matmul
    (ones[1,128] x b_ft[1,512]).
  - PSUM split: 4 banks per (group, side) accumulator, so side/group
    phases pipeline (epilogue of one phase overlaps matmuls of the next).
  - Epilogue: clip(acc,0,1) -> bf16, * W_out half (bf16), reduce -> f32;
    sigmoid(+b_out) on ACT; all DVE traffic in bf16 to halve cost.
  - Inputs the fast path can't handle (values != 1, or a group with
    >32767 unique rows) fall back to a host-side numpy computation.
"""

import sys

sys.path.insert(0, "/opt/trn_rl_repo")

import numpy as np
import ml_dtypes

from concourse import bacc, bass, mybir
from concourse.masks import make_identity
import concourse.tile as tile
from concourse.bass_utils import run_bass_kernel_spmd

P = 128          # SBUF partitions
K = 32           # nnz (active features per position)
J = 8            # batch slots per partition
F = 512          # feature-table output width
NCORES = 8
BPC = P * J      # batch rows per core (1024)
FT_IN = 40960

G = 2            # index groups per core (512 batch rows each)
JG = J // G      # j-slots per group (4)
KC = 2           # k-slots per gather instruction
NI = (BPC // G) * KC   # indices per gather (1024); >1024 hangs the HW ucode
NGPG = K // KC         # gathers per (group, side)
NGATH = G * 2 * NGPG   # total gathers per core
S16 = NI // 16         # columns of the 16-partition-wrapped index tile

GBUFS = 16       # in-flight gather tiles
NQ = 4           # SWDGE descriptor-generation queues
NEARLY = 4       # gathers whose indices load in the first (small) idx DMA
DMA_SCRATCH = 65536   # SWDGE descriptor-ring carveout (bytes/partition)
SINGLE_PACKET = False

f32 = mybir.dt.float32
bf16 = mybir.dt.bfloat16
i16 = mybir.dt.int16
Alu = mybir.AluOpType

TDT = mybir.dt.float8e4            # gathered-table dtype on device
TDT_NP = ml_dtypes.float8_e4m3     # host equivalent
BF16_NP = ml_dtypes.bfloat16
TSCALE = 64.0                      # host premultiplier; PE identity = 1/TSCALE
DR = mybir.MatmulPerfMode.DoubleRow
USE_DR = True                      # fp8 DoubleRow matmuls (2 k-tiles per matmul)


def _build(u_max: int):
    nc = bacc.Bacc("TRN2", target_bir_lowering=False, debug=False,
                   num_devices=NCORES, num_swdge_queues=NQ,
                   dynamic_dma_scratch_size=DMA_SCRATCH)

    tabs = [
        nc.dram_tensor(f"tab{g}", [u_max, F], TDT, kind="ExternalInput")
        for g in range(G)
    ]
    idx_in = nc.dram_tensor("idx", [P, NGATH, S16], i16, kind="ExternalInput")
    bftb = nc.dram_tensor("bftb", [1, F], bf16, kind="ExternalInput")
    w1 = nc.dram_tensor("w1", [P, F], bf16, kind="ExternalInput")
    w2 = nc.dram_tensor("w2", [P, F], bf16, kind="ExternalInput")
    bout = nc.dram_tensor("bout", [P, 1], f32, kind="ExternalInput")
    out = nc.dram_tensor("out", [P, J], f32, kind="ExternalOutput")

    with tile.TileContext(nc) as tc:
        with tc.tile_pool(name="sbuf", bufs=1) as pool, \
             tc.tile_pool(name="gather", bufs=GBUFS) as gpool, \
             tc.tile_pool(name="psum", bufs=1, space="PSUM") as ppool:
            # idx loads split so gather 0 can start as soon as the small
            # first chunk lands; the small epilogue inputs ride the scalar
            # (ACT) HWDGE ring to stay out of the idx DMA's FIFO.
            idx_sb = pool.tile([P, NGATH, S16], i16, tag="idx", name="idx_sb")
            nc.sync.dma_start(out=idx_sb[:, :NEARLY, :], in_=idx_in[:, :NEARLY, :])
            nc.sync.dma_start(out=idx_sb[:, NEARLY:, :], in_=idx_in[:, NEARLY:, :])
            bftb_sb = pool.tile([1, F], bf16, tag="bftb", name="bftb_sb")
            nc.scalar.dma_start(out=bftb_sb[:], in_=bftb[:])
            w_sb = [
                pool.tile([P, F], bf16, tag="w1", name="w1_sb"),
                pool.tile([P, F], bf16, tag="w2", name="w2_sb"),
            ]
            nc.scalar.dma_start(out=w_sb[0][:], in_=w1[:])
            nc.scalar.dma_start(out=w_sb[1][:], in_=w2[:])
            bout_sb = pool.tile([P, 1], f32, tag="bout", name="bout_sb")
            nc.scalar.dma_start(out=bout_sb[:], in_=bout[:])

            ones1 = pool.tile([1, P], bf16, tag="ones1", name="ones1")
            nc.gpsimd.memset(ones1[:], 1.0)

            if USE_DR:
                # stacked identity pair for DoubleRow: lhsT[p, t, m] = (p==m)/TSCALE
                ident2 = pool.tile([P, 2, P], TDT, tag="ident", name="ident2")
                make_identity(nc, ident2[:, 0, :])
                make_identity(nc, ident2[:, 1, :])
            else:
                ident2 = pool.tile([P, P], TDT, tag="ident", name="ident")
                make_identity(nc, ident2[:])
            nc.vector.tensor_scalar_mul(ident2[:], ident2[:], 1.0 / TSCALE)

            def bcast(t2d):  # [P, F] -> [P, JG, F] AP (stride-0 over JG)
                return t2d[:].rearrange("p (j f) -> p j f", j=1).broadcast_to([P, JG, F])

            zp = {}
            for g in range(G):
                for s in range(2):
                    acc = ppool.tile(
                        [P, JG, F], f32, space="PSUM", tag=f"acc{s}",
                        name=f"acc{g}{s}",
                    )
                    # open each chain with the bias: acc[m, f] = 1 * b_ft[f]
                    for jg in range(JG):
                        nc.tensor.matmul(
                            acc[:, jg, :], ones1[:], bftb_sb[:],
                            start=True, stop=False, skip_group_check=True,
                        )
                    for kc in range(NGPG):
                        gi = (g * 2 + s) * NGPG + kc
                        ga = gpool.tile([P, NI // P, F], TDT, tag="g", name=f"ga{gi}")
                        nc.gpsimd.dma_gather(
                            ga[:], tabs[g][:, :], idx_sb[:, gi, :],
                            num_idxs=NI, num_idxs_reg=NI, elem_size=F,
                            queue_num=gi % NQ, single_packet=SINGLE_PACKET,
                        )
                        if USE_DR:
                            for jg in range(JG):
                                for pt in range(KC // 2):
                                    nc.tensor.matmul(
                                        acc[:, jg, :], ident2[:],
                                        ga[:, jg * KC + 2 * pt: jg * KC + 2 * pt + 2, :],
                                        start=False,
                                        stop=(kc == NGPG - 1 and pt == KC // 2 - 1),
                                        perf_mode=DR,
                                        skip_group_check=True,
                                    )
                        else:
                            for jg in range(JG):
                                for t in range(KC):
                                    nc.tensor.matmul(
                                        acc[:, jg, :], ident2[:],
                                        ga[:, jg * KC + t, :],
                                        start=False,
                                        stop=(kc == NGPG - 1 and t == KC - 1),
                                        skip_group_check=True,
                                    )
                    # epilogue: h = clip(acc, 0, 1) (bf16); h *= w_side; zp = sum_f h
                    h = pool.tile([P, JG, F], bf16, tag=f"h{s}", name=f"h{g}{s}")
                    nc.vector.tensor_scalar(
                        out=h[:], in0=acc[:], scalar1=0.0, scalar2=1.0,
                        op0=Alu.max, op1=Alu.min,
                    )
                    nc.vector.tensor_tensor(out=h[:], in0=h[:], in1=bcast(w_sb[s]), op=Alu.mult)
                    zt = pool.tile([P, JG], f32, tag=f"zp{g}{s}", name=f"zp{g}{s}")
                    nc.vector.tensor_reduce(
                        out=zt[:], in_=h[:], axis=mybir.AxisListType.X, op=Alu.add
                    )
                    zp[(g, s)] = zt

            z = pool.tile([P, J], f32, tag="z", name="z")
            for g in range(G):
                nc.vector.tensor_tensor(
                    out=z[:, g * JG:(g + 1) * JG], in0=zp[(g, 0)][:],
                    in1=zp[(g, 1)][:], op=Alu.add,
                )
            out_sb = pool.tile([P, J], f32, tag="out", name="out_sb")
            nc.scalar.activation(
                out=out_sb[:],
                in_=z[:],
                func=mybir.ActivationFunctionType.Sigmoid,
                bias=bout_sb[:, :1],
            )
            nc.sync.dma_start(out=out.ap(), in_=out_sb[:])

    nc.compile()
    return nc


_cache = {}


def _get(u_max: int):
    if u_max not in _cache:
        _cache[u_max] = _build(u_max)
    return _cache[u_max]


def _prep_group(stm_g: np.ndarray, nstm_g: np.ndarray, W8: np.ndarray):
    """Per (core, group): relabel the group's draws by first use (in gather
    scan order), build the compacted fp8 table and wrapped int16 index tiles.

    stm_g/nstm_g: [512, 32] int32 (group batch rows, core-local order
    r = jg*128 + p). Returns (table [U, F] fp8, idx [2*NGPG, 128, S16] i16, U).
    """
    arrs = []
    for A in (stm_g, nstm_g):
        # [jg, p, kc, t] -> [kc, jg, t, p]; flatten -> gather position order
        B = A.reshape(JG, P, NGPG, KC).transpose(2, 0, 3, 1)
        arrs.append(B.reshape(NGPG, NI))
    S = np.concatenate(arrs, axis=0)          # [2*NGPG, NI] in scan order
    flat = S.ravel().astype(np.int64)
    u, first = np.unique(flat, return_index=True)
    order = np.argsort(first)                 # first-use order of unique rows
    rank = np.empty(len(u), dtype=np.int64)
    rank[order] = np.arange(len(u))
    labels = rank[np.searchsorted(u, flat)]
    if len(u) > 32767:
        return None, None, len(u)
    tab = W8[u[order]]                        # [U, F] fp8
    idx16 = labels.astype(np.int16).reshape(2 * NGPG, S16, 16)
    idxw = np.ascontiguousarray(
        np.tile(idx16.transpose(0, 2, 1), (1, J, 1))
    )                                         # [2*NGPG, 128, S16]
    return tab, idxw, len(u)


def _numpy_fallback(stm_indices, nstm_indices, values, W_ft, b_ft, W_out, b_out):
    stm_ft = np.einsum("bk,bkf->bf", values, W_ft[stm_indices]) + b_ft
    nstm_ft = np.einsum("bk,bkf->bf", values, W_ft[nstm_indices]) + b_ft
    hidden = np.clip(np.concatenate([stm_ft, nstm_ft], axis=1), 0.0, 1.0)
    zv = hidden @ W_out + b_out
    return (1.0 / (1.0 + np.exp(-zv))).astype(np.float32)


def kernel(stm_indices, nstm_indices, values, W_ft, b_ft, W_out, b_out, _trace=False):
    stm_indices = np.asarray(stm_indices)
    nstm_indices = np.asarray(nstm_indices)
    values = np.asarray(values, dtype=np.float32)
    W_ft = np.ascontiguousarray(np.asarray(W_ft, dtype=np.float32))
    b_ft = np.asarray(b_ft, dtype=np.float32)
    W_out = np.asarray(W_out, dtype=np.float32)
    b_out = np.asarray(b_out, dtype=np.float32)

    if not bool(np.all(values == 1.0)):
        r = _numpy_fallback(stm_indices, nstm_indices, values, W_ft, b_ft,
                            W_out, b_out)
        return (r, None) if _trace else r

    W8 = (W_ft * TSCALE).astype(TDT_NP)

    tabs = []   # [core][group] -> table
    idxs = []   # [core] -> [NGATH, 128, S16]
    u_max = 0
    for c in range(NCORES):
        ct, ci = [], []
        for g in range(G):
            sl = slice(c * BPC + g * (BPC // G), c * BPC + (g + 1) * (BPC // G))
            tab, idxw, u = _prep_group(stm_indices[sl], nstm_indices[sl], W8)
            if tab is None:
                r = _numpy_fallback(stm_indices, nstm_indices, values, W_ft,
                                    b_ft, W_out, b_out)
                return (r, None) if _trace else r
            ct.append(tab)
            ci.append(idxw)
            u_max = max(u_max, u)
        tabs.append(ct)
        idxs.append(np.concatenate(ci, axis=0))

    u_pad = -(-u_max // 1024) * 1024
    nc = _get(u_pad)

    bftb_rep = np.ascontiguousarray(b_ft.astype(BF16_NP).reshape(1, F))
    w1_rep = np.ascontiguousarray(
        np.broadcast_to(W_out[:F, 0], (P, F)).astype(BF16_NP))
    w2_rep = np.ascontiguousarray(
        np.broadcast_to(W_out[F:, 0], (P, F)).astype(BF16_NP))
    bout_rep = np.full((P, 1), b_out[0], dtype=np.float32)

    in_maps = []
    for c in range(NCORES):
        m = {
            "idx": np.ascontiguousarray(idxs[c].transpose(1, 0, 2)),  # [P, NGATH, S16]
            "bftb": bftb_rep,
            "w1": w1_rep,
            "w2": w2_rep,
            "bout": bout_rep,
        }
        for g in range(G):
            tfull = np.zeros((u_pad, F), dtype=TDT_NP)
            tfull[:tabs[c][g].shape[0]] = tabs[c][g]
            m[f"tab{g}"] = tfull
        in_maps.append(m)

    res = run_bass_kernel_spmd(
        nc, in_maps, core_ids=list(range(NCORES)), trace=_trace
    )
    # out[p, j] holds batch row j*128 + p
    out = np.concatenate(
        [res.results[c]["out"].T.reshape(BPC) for c in range(NCORES)]
    ).reshape(8192, 1)
    if _trace:
        return out, res
    return out
